# revision 12
# baseline (speedup 1.0000x reference)
"""Trainium2 Bass kernel for nn_DialogueGCNModel (DialogueGCN forward).

Strategy (data-parallel over dialogues, 4 dialogues per core):
  - Edges never cross dialogues, so RGCN scatter/gather becomes dense
    per-dialogue adjacency matmuls.  All large matmuls run in fp8-e4m3
    DoubleRow mode (two 128-deep contraction slices per pass); the
    adjacency pairs adjacent relations (same src tile, same band window).
    Weights are pre-scaled on the host (x64 / x256) to stay in fp8 normal
    range; scales are unwound in the psum->sbuf copies.
  - 1/deg is folded into the adjacency masks on the host; masks are
    band-cropped to the +-10 edge window; the root-weight matmul runs first
    in the same PSUM group and zeroes it.
  - M^T is padded with a constant-ones row so b_t rides along as one more
    contraction row; padded w_t/w_lin rows are zero.
  - Softmaxes skip the running max (tanh <= 1; logits ~ +-0.1); the final
    log-softmax computes Ln via a 3-term series, so every ACT function
    (Tanh/Exp/Identity/Relu) lives in one table set: no mid-kernel reloads.
  - psum->SBUF copies are batched in pairs (two matmul chains share one
    [128,512] psum bank, one copy) and split across ACT and DVE; Pool
    (gpsimd) takes all SBUF-only elementwise work (memsets, alpha scaling,
    the Ln series) since it cannot touch PSUM.
  - The whole back half is pipelined per dialogue-pair: stage2/3 for a pair
    run while the next pair's adjacency masks stream in; stage5 -> scores ->
    softmax -> attention tail are interleaved per dialogue so the ACT/DVE
    softmax chains hide under the next dialogue's matmuls.

kernel(**inputs) takes FULL inputs, runs 8-core SPMD via
bass_utils.run_bass_kernel_spmd, returns the FULL (8192, 7) f32 output.
"""

import numpy as np
import ml_dtypes

BF16 = ml_dtypes.bfloat16
FP8 = ml_dtypes.float8_e4m3

# Problem constants (hardcoded per contract)
B, L, D, H, R, NB, C = 32, 256, 1024, 128, 8, 30, 7
MEM = D + H            # 1152
N = B * L              # 8192
NCORES = 8
DPC = B // NCORES      # dialogues per core = 4
NLOC = DPC * L         # nodes per core = 1024
NT = NLOC // 128       # node tiles per core = 8
KT = D // 128          # contraction tiles over D = 8
MT = MEM // 128        # tiles over MEM = 9
MTP = MT + 1           # padded to even for DoubleRow pairing

AW = 144               # adjacency band width (window +-10 fits in 144)
ALO = (0, 112)         # per-src-tile dst column offset of the band

N_WARM = 16            # warm-up matmuls covering the DMA lead-in

_cache = {}


def _build_program(use_mask):
    import concourse.bacc as bacc
    import concourse.tile as tile
    import concourse.mybir as mybir
    from concourse.masks import make_identity

    dt = mybir.dt
    f32, bf16, fp8 = dt.float32, dt.bfloat16, dt.float8e4
    AF = mybir.ActivationFunctionType
    OP = mybir.AluOpType
    DR = mybir.MatmulPerfMode.DoubleRow

    nc = bacc.Bacc("TRN2", target_bir_lowering=False, debug=False,
                   num_devices=NCORES)

    dram = nc.dram_tensor
    # all pre-packed on host to [128, ...] SBUF layout (contiguous lines)
    xt_d = dram("xt", [128, DPC * KT * L], fp8, kind="ExternalInput")
    wrel_d = dram("wrel", [128, 2 * KT * 512], fp8, kind="ExternalInput")
    wr1_d = dram("wr1", [128, KT * H], fp8, kind="ExternalInput")
    at_d = dram("at", [128, DPC * R * 2 * AW], fp8, kind="ExternalInput")
    bt_d = dram("bt", [128, DPC * 2 * L], fp8, kind="ExternalInput")
    w2_d = dram("w2", [128, 2 * H], fp8, kind="ExternalInput")
    wt_d = dram("wt", [128, MT * MTP * 128], fp8, kind="ExternalInput")
    wlin_d = dram("wlin", [128, MTP * H], fp8, kind="ExternalInput")
    wfc_d = dram("wfc", [128, C], bf16, kind="ExternalInput")
    bias_d = dram("bias", [128, 12], f32, kind="ExternalInput")
    bfc_d = dram("bfc", [1, C], bf16, kind="ExternalInput")
    if use_mask:
        um_d = dram("um", [DPC, 2, L], f32, kind="ExternalInput")
    out_d = dram("out", [NLOC, C], f32, kind="ExternalOutput")

    with tile.TileContext(nc) as tc:
        from contextlib import ExitStack
        with ExitStack() as ctx:
            consts = ctx.enter_context(tc.tile_pool(name="consts", bufs=1))
            big = ctx.enter_context(tc.tile_pool(name="big", bufs=1))
            work = ctx.enter_context(tc.tile_pool(name="work", bufs=6))
            ps = ctx.enter_context(tc.tile_pool(name="ps", bufs=6, space="PSUM"))
            pst = ctx.enter_context(tc.tile_pool(name="pst", bufs=2, space="PSUM"))

            dma_a = nc.sync.dma_start      # queue A: PE-critical operands
            dma_b = nc.scalar.dma_start    # queue B: small tensors
            mm = nc.tensor.matmul

            # ---- persistent operands; DMAs chunked + ordered by first use ----
            xt = consts.tile([128, DPC, KT, L], fp8)     # d-major
            wrel = consts.tile([128, 2, KT, 512], fp8)   # h2-major
            wr1 = consts.tile([128, KT, H], fp8)
            at = consts.tile([128, DPC, R, 2, AW], fp8)
            bt = consts.tile([128, DPC, 2, L], fp8)
            wt = consts.tile([128, MT, MTP, 128], fp8)   # n2-major
            wlin = consts.tile([128, MTP, H], fp8)

            XTW = KT * L
            def dma_xt(d):
                dma_a(out=xt[:, d], in_=xt_d[:, d * XTW:(d + 1) * XTW])

            dma_a(out=wrel[:, 0, 0:4, :], in_=wrel_d[:, 0:2048])
            dma_xt(0)
            dma_a(out=wrel[:, 1, 0:4, :], in_=wrel_d[:, 4096:6144])
            dma_a(out=wrel[:, 0, 4:8, :], in_=wrel_d[:, 2048:4096])
            dma_a(out=wrel[:, 1, 4:8, :], in_=wrel_d[:, 6144:8192])
            dma_xt(1)
            dma_a(out=wr1, in_=wr1_d[:])
            dma_xt(2)
            dma_xt(3)
            ATW = R * 2 * AW
            for d in range(2):
                dma_a(out=at[:, d], in_=at_d[:, d * ATW:(d + 1) * ATW])
            dma_a(out=bt, in_=bt_d[:])
            for c0 in range(0, MT, 2):    # wt n2-pair chunks (4x2 + 1)
                c1 = min(c0 + 2, MT)
                dma_a(out=wt[:, c0:c1], in_=wt_d[:, c0 * MTP * 128:
                                              c1 * MTP * 128])
            for d in range(2, DPC):
                dma_a(out=at[:, d], in_=at_d[:, d * ATW:(d + 1) * ATW])
            dma_a(out=wlin, in_=wlin_d[:])
            w2 = consts.tile([128, 2, H], fp8)
            dma_b(out=w2, in_=w2_d[:])
            wfc = consts.tile([128, C], bf16)
            dma_b(out=wfc, in_=wfc_d[:])
            bias = consts.tile([128, 12], f32)
            dma_b(out=bias, in_=bias_d[:])
            bfc = consts.tile([1, C], bf16)
            dma_b(out=bfc, in_=bfc_d[:])
            if use_mask:
                import concourse.bass as bass
                um = consts.tile([128, DPC, 2, L], f32)
                src = um_d[:]
                bc = bass.AP(tensor=src.tensor, offset=src.offset,
                             ap=[[0, 128]] + list(src.ap))
                nc.gpsimd.dma_start(out=um, in_=bc)

            # small constants + memsets on Pool (ACT/DVE stay free)
            wz = consts.tile([128, 2, 512], fp8)
            nc.gpsimd.memset(wz, 0.0)
            ones_row = consts.tile([1, 128], bf16)
            nc.gpsimd.memset(ones_row, 1.0)
            neg1 = consts.tile([128, 1], f32)
            nc.gpsimd.memset(neg1, -1.0)
            ident = consts.tile([128, 128], bf16)
            make_identity(nc, ident)

            # persistent activation/state tiles (slot-major for d-contiguity)
            xr = consts.tile([128, NT, R, H], fp8)       # 16 * xr
            out1T = consts.tile([128, DPC, L], bf16)
            out18 = consts.tile([128, NT, H], fp8)
            nbout = consts.tile([128, 2, DPC, L], fp8)   # [nbT; out1T]
            out2x = consts.tile([128, 2, DPC, L], fp8)   # [out2T; ONES]
            hidT = consts.tile([128, DPC, L], bf16)
            G8 = consts.tile([128, DPC, 2, H], fp8)
            alphaT = consts.tile([128, DPC, 2, 2 * 128], fp8)  # 64*alpha^T
            s_all = consts.tile([128, 16], f32)
            o_all = consts.tile([128, DPC, 2, 8], f32)
            v_t = consts.tile([128, 16], f32)
            h_t = consts.tile([128, 16], f32)
            # ones row of M^T: pairs the 64*b_t row of w_t (bias via matmul)
            nc.gpsimd.memset(out2x[:, 1], 1.0)

            XcTs = []
            for d in range(DPC):
                XcT = big.tile([128, MTP, L], fp8, tag=f"XcT{d}")
                nc.gpsimd.memset(XcT[:, MT, :], 0.0)
                XcTs.append(XcT)

            # warm-up: dependency-free DR matmuls keep the PE p-state ramped
            # during the DMA lead-in; `warm` psum is never read.
            warm = ps.tile([128, 512], f32, tag="mm")
            for _ in range(N_WARM):
                mm(warm[:, :256], lhsT=wz[:, :, :128], rhs=wz[:, :, :256],
                   start=True, stop=True, perf_mode=DR, skip_group_check=True)

            # psum->SBUF copy on ACT (eng 0) or DVE (eng 1)
            def e_copy(eng, out, in_, scale=None):
                if eng == 0:
                    nc.scalar.activation(out, in_, AF.Identity,
                                         scale=(scale or 1.0))
                elif scale is not None:
                    nc.vector.tensor_scalar_mul(out, in_, scale)
                else:
                    nc.vector.tensor_copy(out, in_)

            # M^T subtile-pair accessor (m2 in 0..4): xt pairs, then
            # [out2T; ones]
            def rhs_pair(m2, d):
                if m2 < 4:
                    return xt[:, d, 2 * m2:2 * m2 + 2, :]
                return out2x[:, :, d, :]

            def lhs_pair(m2, d, st):
                if m2 < 4:
                    return xt[:, d, 2 * m2:2 * m2 + 2,
                              st * 128:(st + 1) * 128]
                return out2x[:, :, d, st * 128:st * 128 + 128]

            # ---- stage 1: xr = x @ w_rel; chain-major, per-dialogue waves --
            ci = 0
            for d in range(DPC):
                chains = []
                for h2 in range(2):
                    for st in range(2):
                        p_c = ps.tile([128, 512], f32, tag="mm")
                        chains.append((h2, st, p_c))
                for h2, st, p in chains:
                    for k2 in range(4):
                        mm(p, lhsT=xt[:, d, 2 * k2:2 * k2 + 2,
                                      st * 128:(st + 1) * 128],
                           rhs=wrel[:, h2, 2 * k2:2 * k2 + 2, :],
                           start=(k2 == 0), stop=(k2 == 3), perf_mode=DR,
                           skip_group_check=True)
                for h2, st, p in chains:
                    i = 2 * d + st
                    e_copy(ci % 2, xr[:, i, 4 * h2:4 * h2 + 4, :], p,
                           scale=1.0 / 16.0)
                    ci += 1

            def stage2_q(q):
                # root (DR, zeroes psum) + banded adjacency (relation-pair DR)
                pa = ps.tile([128, 512], f32, tag="mm")
                for j in range(2):
                    d = 2 * q + j
                    o = j * 256
                    for k2 in range(4):
                        mm(pa[:, o:o + L], lhsT=wr1[:, 2 * k2:2 * k2 + 2, :],
                           rhs=xt[:, d, 2 * k2:2 * k2 + 2, :],
                           start=(k2 == 0), stop=False, perf_mode=DR,
                           skip_group_check=True)
                    bi = 0
                    for ri in range(R // 2):
                        for st in range(2):
                            bi += 1
                            mm(pa[:, o + ALO[st]:o + ALO[st] + AW],
                               lhsT=xr[:, 2 * d + st, 2 * ri:2 * ri + 2, :],
                               rhs=at[:, d, 2 * ri:2 * ri + 2, st, :],
                               start=False, stop=(bi == R), perf_mode=DR,
                               skip_group_check=True)
                nc.scalar.activation(out1T[:, 2 * q:2 * q + 2, :], pa,
                                     AF.Identity, scale=1.0 / 64.0,
                                     bias=bias[:, 0:1])
                nc.gpsimd.tensor_copy(nbout[:, 1, 2 * q:2 * q + 2, :],
                                      out1T[:, 2 * q:2 * q + 2, :])
                for j in range(2):
                    d = 2 * q + j
                    tp = pst.tile([128, 256], bf16, tag="tr")
                    for st in range(2):
                        nc.tensor.transpose(
                            tp[:, st * 128:(st + 1) * 128],
                            out1T[:, d, st * 128:(st + 1) * 128], ident)
                    nc.vector.tensor_copy(out18[:, 2 * d:2 * d + 2, :], tp)

            def stage3_q(q):
                p2 = ps.tile([128, 512], f32, tag="mm")
                for j in range(2):
                    d = 2 * q + j
                    mm(p2[:, j * 256:j * 256 + L],
                       lhsT=out18[:, 2 * d:2 * d + 2, :],
                       rhs=bt[:, d, :, :], start=True, stop=True,
                       perf_mode=DR, skip_group_check=True)
                nc.vector.tensor_copy(nbout[:, 0, 2 * q:2 * q + 2, :], p2)
                p3 = ps.tile([128, 512], f32, tag="mm")
                for j in range(2):
                    d = 2 * q + j
                    mm(p3[:, j * 256:j * 256 + L], lhsT=w2,
                       rhs=nbout[:, :, d, :], start=True, stop=True,
                       perf_mode=DR, skip_group_check=True)
                nc.scalar.activation(out2x[:, 0, 2 * q:2 * q + 2, :], p3,
                                     AF.Identity, scale=1.0 / 64.0,
                                     bias=bias[:, 1:2])

            def stage5_d(d):
                # Xc^T = w_t^T M^T (+ b_t via ones row); paired n2 chains
                for n2p in range(4):
                    p4 = ps.tile([128, 512], f32, tag="mm")
                    for j in range(2):
                        n2 = 2 * n2p + j
                        for m2 in range(5):
                            mm(p4[:, j * 256:j * 256 + L],
                               lhsT=wt[:, n2, 2 * m2:2 * m2 + 2, :],
                               rhs=rhs_pair(m2, d), start=(m2 == 0),
                               stop=(m2 == 4), perf_mode=DR,
                               skip_group_check=True)
                    e_copy(0 if n2p == 0 else 1,
                           XcTs[d][:, 2 * n2p:2 * n2p + 2, :], p4)
                p4 = ps.tile([128, 512], f32, tag="mm")
                for m2 in range(5):
                    mm(p4[:, :L], lhsT=wt[:, MT - 1, 2 * m2:2 * m2 + 2, :],
                       rhs=rhs_pair(m2, d), start=(m2 == 0), stop=(m2 == 4),
                       perf_mode=DR)
                e_copy(0, XcTs[d][:, MT - 1, :], p4[:, :L])

            alfs = {}

            def scores_d(d):
                for tt in range(2):
                    p5 = ps.tile([128, 512], f32, tag="mm")
                    for n2 in range(5):
                        mm(p5[:, :L],
                           lhsT=XcTs[d][:, 2 * n2:2 * n2 + 2,
                                        tt * 128:(tt + 1) * 128],
                           rhs=rhs_pair(n2, d), start=(n2 == 0),
                           stop=(n2 == 4), perf_mode=DR)
                    z = big.tile([128, L], f32, tag=f"z{d}{tt}")
                    if use_mask:
                        # um slot0 = um^2/64 -> z_in = scores*um^2
                        nc.vector.tensor_mul(z, p5[:, :L], um[:, d, 0, :])
                        nc.scalar.activation(z, z, AF.Tanh)
                    else:
                        nc.scalar.activation(z, p5[:, :L], AF.Tanh,
                                             scale=1.0 / 64.0)
                    ssum = work.tile([128, 1], f32, tag=f"ssum{d}{tt}")
                    # tanh <= 1, so exp(z - 1) is safe without a max pass
                    nc.scalar.activation(z, z, AF.Exp, bias=neg1,
                                         accum_out=ssum)
                    if use_mask:
                        nc.vector.tensor_mul(z, z, um[:, d, 1, :])
                        nc.vector.reduce_sum(out=ssum, in_=z,
                                             axis=mybir.AxisListType.X)
                    rinv = work.tile([128, 1], f32, tag=f"rinv{d}{tt}")
                    nc.vector.reciprocal(rinv, ssum)
                    alf = big.tile([128, L], bf16, tag=f"alf{d}{tt}")
                    # 64*alpha on Pool (SBUF-only engine)
                    nc.gpsimd.tensor_scalar(out=alf, in0=z, scalar1=rinv,
                                            scalar2=64.0, op0=OP.mult,
                                            op1=OP.mult)
                    alfs[(d, tt)] = alf

            def g_d(d):
                pg = ps.tile([128, 512], f32, tag="mm")
                for st in range(2):
                    for m2 in range(5):
                        mm(pg[:, st * 128:(st + 1) * 128],
                           lhsT=lhs_pair(m2, d, st),
                           rhs=wlin[:, 2 * m2:2 * m2 + 2, :],
                           start=(m2 == 0), stop=(m2 == 4), perf_mode=DR,
                           skip_group_check=True)
                nc.vector.tensor_scalar_mul(G8[:, d, :, :], pg[:, :256],
                                            1.0 / 64.0)

            out_ap = out_d[:].rearrange("(d tt p) c -> p d tt c", d=DPC, tt=2)
            LN7 = float(np.log(7.0))

            def tail_d(d):
                # all four alpha transposes share one psum bank, one copy
                tp = pst.tile([128, 512], bf16, tag="tr")
                for st in range(2):
                    for tt in range(2):
                        nc.tensor.transpose(
                            tp[:, st * 256 + tt * 128:st * 256 + tt * 128 + 128],
                            alfs[(d, tt)][:, st * 128:(st + 1) * 128], ident)
                nc.vector.tensor_copy(alphaT[:, d, :, :], tp)
                p7 = ps.tile([128, 512], f32, tag="mm")
                mm(p7[:, :L], lhsT=G8[:, d, :, :], rhs=alphaT[:, d, :, :],
                   start=True, stop=True, perf_mode=DR)
                nc.scalar.activation(hidT[:, d, :], p7[:, :L], AF.Relu,
                                     scale=1.0 / 64.0, bias=bias[:, 11:12])
                p8 = ps.tile([128, 512], f32, tag="mm")
                for tt in range(2):
                    o = tt * 8
                    mm(p8[:, o:o + C],
                       lhsT=hidT[:, d, tt * 128:(tt + 1) * 128],
                       rhs=wfc, start=True, stop=False,
                       skip_group_check=True)
                    mm(p8[:, o:o + C], lhsT=ones_row, rhs=bfc, start=False,
                       stop=True, skip_group_check=True)
                for tt in range(2):
                    idx = d * 2 + tt
                    e8 = work.tile([128, 8], f32, tag="e8")
                    nc.scalar.activation(e8[:, :C], p8[:, tt * 8:tt * 8 + C],
                                         AF.Exp,
                                         accum_out=s_all[:, idx:idx + 1])
                nc.vector.tensor_copy(o_all[:, d, :, :], p8[:, :16])
                # per-dialogue log-softmax close-out + output DMA (Pool):
                # ln(s) = ln7 + v - v^2/2 + v^3/3, v = s/7 - 1 (|v| < 0.1)
                i0 = 2 * d
                sv = v_t[:, i0:i0 + 2]
                sh = h_t[:, i0:i0 + 2]
                nc.gpsimd.tensor_scalar(out=sv, in0=s_all[:, i0:i0 + 2],
                                        scalar1=1.0 / 7.0, scalar2=-1.0,
                                        op0=OP.mult, op1=OP.add)
                nc.gpsimd.tensor_scalar(out=sh, in0=sv, scalar1=1.0 / 3.0,
                                        scalar2=-0.5, op0=OP.mult, op1=OP.add)
                nc.gpsimd.tensor_mul(sh, sh, sv)
                nc.gpsimd.tensor_scalar_add(sh, sh, 1.0)
                nc.gpsimd.tensor_mul(sh, sh, sv)   # h = ln(s) - ln7
                for tt in range(2):
                    idx = d * 2 + tt
                    nc.gpsimd.tensor_scalar(
                        out=o_all[:, d, tt, :C], in0=o_all[:, d, tt, :C],
                        scalar1=h_t[:, idx:idx + 1], scalar2=-LN7,
                        op0=OP.subtract, op1=OP.add)
                dma_a(out=out_ap[:, d], in_=o_all[:, d, :, 0:C])

            # ---- pipelined back half over dialogue pairs; scores-d emitted
            # after G-d / tail-(d-1) so XcT copies hide under PE work ----
            for q in range(2):
                stage2_q(q)
                stage3_q(q)
                for d in (2 * q, 2 * q + 1):
                    stage5_d(d)
                    g_d(d)
                    if d > 0:
                        tail_d(d - 1)
                    scores_d(d)
            tail_d(DPC - 1)

    nc.compile()
    return nc


def prep_inputs(x, edge_src, edge_dst, edge_type, umask, basis, comp,
                w_root1, b1, w_rel2, b_rel2, w_root2, w_t, b_t,
                w_lin, b_lin, w_fc, b_fc):
    """Host-side sharding / packing into the device's [128, ...] layouts."""
    x = np.asarray(x, np.float32)
    src = np.asarray(edge_src, np.int64)
    dst = np.asarray(edge_dst, np.int64)
    ety = np.asarray(edge_type, np.int64)
    umask = np.asarray(umask, np.float32)
    basis = np.asarray(basis, np.float32)
    comp = np.asarray(comp, np.float32)

    g_s = src // L
    assert np.array_equal(g_s, dst // L), "edges must stay within a dialogue"

    # w_rel[r] = sum_b comp[r,b] basis[b]; packed h2-major:
    # wrel[p, h2, k, j] = 256 * w_rel[r, 128k+p, h], (r, h) = divmod(512*h2+j, H)
    w_rel = np.einsum('rb,bdh->rdh', comp, basis)          # (R, D, H)
    wr = (256.0 * w_rel).transpose(1, 0, 2).reshape(D, R * H)   # [d, rH]
    wr = wr.reshape(KT, 128, 2, 512).transpose(1, 2, 0, 3)      # [p, h2, k, 512]
    wrel_pack = np.ascontiguousarray(wr.reshape(128, 2 * KT * 512)).astype(FP8)

    deg = np.bincount(dst, minlength=N).astype(np.float64)
    inv_deg = np.where(deg > 0, 1.0 / np.maximum(deg, 1), 0.0).astype(np.float32)

    at_all = np.zeros((B, R, L, L), np.float32)   # [dlg, r, src, dst]
    ls, ld = src % L, dst % L
    np.add.at(at_all, (g_s, ety, ls, ld), 1.0)
    bt_all = np.zeros((B, L, L), np.float32)
    np.add.at(bt_all, (g_s, ls, ld), 1.0)
    # fold 4/deg into the relational masks (device pa = 64*(agg+root))
    at_all *= 4.0 * inv_deg.reshape(B, 1, 1, L)

    # band-crop: src tile st covers dst cols [ALO[st], ALO[st]+AW)
    at_band = np.zeros((B, R, 2, 128, AW), np.float32)
    for st in range(2):
        at_band[:, :, st] = at_all[:, :, st * 128:(st + 1) * 128,
                                   ALO[st]:ALO[st] + AW]
    assert np.isclose(at_band.sum(), at_all.sum()), \
        "edges outside the adjacency band"

    use_mask = not bool(np.all(umask == 1.0))

    bias_pack = np.zeros((128, 12), np.float32)
    bias_pack[:, 0] = np.asarray(b1, np.float32)
    bias_pack[:, 1] = np.asarray(b_rel2, np.float32)
    bias_pack[:, 11] = np.asarray(b_lin, np.float32)

    def pack_k(w, scale):   # [K*128, F] -> [128, KT', F]
        k = w.shape[0] // 128
        return np.ascontiguousarray(
            (scale * np.asarray(w, np.float32)).reshape(k, 128, -1)
            .transpose(1, 0, 2).reshape(128, -1)).astype(FP8)

    # wt n2-major: wt[p, n2, m, j] = 64*w_t[128m+p, 128n2+j];
    # row 1152 (m=9, p=0) = 64*b_t, pairing the ones row of M^T
    wt_pad = np.zeros((MTP * 128, MEM), np.float32)
    wt_pad[:MEM] = 64.0 * np.asarray(w_t, np.float32)
    wt_pad[MEM] = 64.0 * np.asarray(b_t, np.float32)
    wt9 = wt_pad.reshape(MTP, 128, MT, 128).transpose(1, 2, 0, 3)
    wt_pack = np.ascontiguousarray(wt9.reshape(128, -1)).astype(FP8)

    wlin_pad = np.zeros((MTP * 128, H), np.float32)
    wlin_pad[:MEM] = np.asarray(w_lin, np.float32)
    w2_stack = np.stack([np.asarray(w_rel2, np.float32),
                         np.asarray(w_root2, np.float32)], axis=1)  # [H,2,H]

    shared = {
        "wrel": wrel_pack,
        "wr1": pack_k(np.asarray(w_root1, np.float32), 64.0),
        "w2": np.ascontiguousarray(
            (64.0 * w2_stack).reshape(128, 2 * H)).astype(FP8),
        "wt": wt_pack,
        "wlin": pack_k(wlin_pad, 64.0),
        "wfc": np.asarray(w_fc, np.float32).astype(BF16),
        "bias": bias_pack,
        "bfc": np.asarray(b_fc, np.float32).reshape(1, C).astype(BF16),
    }

    in_maps = []
    for c in range(NCORES):
        m = dict(shared)
        xl = x[c * NLOC:(c + 1) * NLOC]           # (1024, 1024)
        # xt d-major: xt[p, d, k, j] = x[d*256+j, 128k+p]
        xtd = xl.T.reshape(KT, 128, DPC, L).transpose(1, 2, 0, 3)
        m["xt"] = np.ascontiguousarray(xtd.reshape(128, -1)).astype(FP8)
        atc = at_band[c * DPC:(c + 1) * DPC]      # (DPC, R, 2, 128, AW)
        m["at"] = np.ascontiguousarray(
            atc.transpose(3, 0, 1, 2, 4).reshape(128, -1)).astype(FP8)
        btc = bt_all[c * DPC:(c + 1) * DPC]
        m["bt"] = np.ascontiguousarray(
            btc.reshape(DPC, 2, 128, L).transpose(2, 0, 1, 3)
            .reshape(128, -1)).astype(FP8)
        if use_mask:
            uml = umask[c * DPC:(c + 1) * DPC]    # (DPC, L)
            m["um"] = np.stack([uml * uml / 64.0, uml], axis=1
                               ).astype(np.float32)
        in_maps.append(m)
    return in_maps, use_mask


_last_results = None


def kernel(**inputs):
    global _last_results
    from concourse.bass_utils import run_bass_kernel_spmd

    in_maps, use_mask = prep_inputs(**inputs)
    if use_mask not in _cache:
        _cache[use_mask] = _build_program(use_mask)
    nc = _cache[use_mask]
    res = run_bass_kernel_spmd(nc, in_maps, core_ids=list(range(NCORES)))
    _last_results = res
    return np.concatenate([res.results[c]["out"] for c in range(NCORES)],
                          axis=0)


# revision 13
# speedup vs baseline: 1.0698x; 1.0698x over previous
"""Trainium2 Bass kernel for nn_DialogueGCNModel (DialogueGCN forward).

Strategy (data-parallel over dialogues, 4 dialogues per core):
  - Edges never cross dialogues, so RGCN scatter/gather becomes dense
    per-dialogue adjacency matmuls.  All large matmuls run in fp8-e4m3
    DoubleRow mode (two 128-deep contraction slices per pass); the
    adjacency pairs adjacent relations (same src tile, same band window).
    Weights are pre-scaled on the host (x64 / x256) to stay in fp8 normal
    range; scales are unwound in the psum->sbuf copies.
  - 1/deg is folded into the adjacency masks on the host; masks are
    band-cropped to the +-10 edge window; the root-weight matmul runs first
    in the same PSUM group and zeroes it.
  - M^T is padded with a constant-ones row so b_t rides along as one more
    contraction row; padded w_t/w_lin rows are zero.
  - Softmaxes skip the running max (tanh <= 1; logits ~ +-0.1); the final
    log-softmax computes Ln via a 3-term series, so every ACT function
    (Tanh/Exp/Identity/Relu) lives in one table set: no mid-kernel reloads.
  - psum->SBUF copies are batched in pairs (two matmul chains share one
    [128,512] psum bank, one copy) and split across ACT and DVE; Pool
    (gpsimd) takes all SBUF-only elementwise work (memsets, alpha scaling,
    the Ln series) since it cannot touch PSUM.
  - The whole back half is pipelined per dialogue-pair: stage2/3 for a pair
    run while the next pair's adjacency masks stream in; stage5 -> scores ->
    softmax -> attention tail are interleaved per dialogue so the ACT/DVE
    softmax chains hide under the next dialogue's matmuls.

kernel(**inputs) takes FULL inputs, runs 8-core SPMD via
bass_utils.run_bass_kernel_spmd, returns the FULL (8192, 7) f32 output.
"""

import numpy as np
import ml_dtypes

BF16 = ml_dtypes.bfloat16
FP8 = ml_dtypes.float8_e4m3

# Problem constants (hardcoded per contract)
B, L, D, H, R, NB, C = 32, 256, 1024, 128, 8, 30, 7
MEM = D + H            # 1152
N = B * L              # 8192
NCORES = 8
DPC = B // NCORES      # dialogues per core = 4
NLOC = DPC * L         # nodes per core = 1024
NT = NLOC // 128       # node tiles per core = 8
KT = D // 128          # contraction tiles over D = 8
MT = MEM // 128        # tiles over MEM = 9
MTP = MT + 1           # padded to even for DoubleRow pairing

AW = 144               # adjacency band width (window +-10 fits in 144)
ALO = (0, 112)         # per-src-tile dst column offset of the band

N_WARM = 16            # warm-up matmuls covering the DMA lead-in

_cache = {}


def _build_program(use_mask):
    import concourse.bacc as bacc
    import concourse.tile as tile
    import concourse.mybir as mybir
    from concourse.masks import make_identity

    dt = mybir.dt
    f32, bf16, fp8 = dt.float32, dt.bfloat16, dt.float8e4
    AF = mybir.ActivationFunctionType
    OP = mybir.AluOpType
    DR = mybir.MatmulPerfMode.DoubleRow

    nc = bacc.Bacc("TRN2", target_bir_lowering=False, debug=False,
                   num_devices=NCORES)

    dram = nc.dram_tensor
    # all pre-packed on host to [128, ...] SBUF layout (contiguous lines)
    xt_d = dram("xt", [128, DPC * KT * L], fp8, kind="ExternalInput")
    wrel_d = dram("wrel", [128, 2 * KT * 512], fp8, kind="ExternalInput")
    wr1_d = dram("wr1", [128, KT * H], fp8, kind="ExternalInput")
    at_d = dram("at", [128, DPC * R * 2 * AW], fp8, kind="ExternalInput")
    bt_d = dram("bt", [128, DPC * 2 * L], fp8, kind="ExternalInput")
    w2_d = dram("w2", [128, 2 * H], fp8, kind="ExternalInput")
    wt_d = dram("wt", [128, MT * MTP * 128], fp8, kind="ExternalInput")
    wlin_d = dram("wlin", [128, MTP * H], fp8, kind="ExternalInput")
    wfc_d = dram("wfc", [128, C], bf16, kind="ExternalInput")
    bias_d = dram("bias", [128, 12], f32, kind="ExternalInput")
    bfc_d = dram("bfc", [1, C], bf16, kind="ExternalInput")
    if use_mask:
        um_d = dram("um", [DPC, 2, L], f32, kind="ExternalInput")
    out_d = dram("out", [NLOC, C], f32, kind="ExternalOutput")

    with tile.TileContext(nc) as tc:
        from contextlib import ExitStack
        with ExitStack() as ctx:
            consts = ctx.enter_context(tc.tile_pool(name="consts", bufs=1))
            big = ctx.enter_context(tc.tile_pool(name="big", bufs=1))
            work = ctx.enter_context(tc.tile_pool(name="work", bufs=6))
            ps = ctx.enter_context(tc.tile_pool(name="ps", bufs=6, space="PSUM"))
            pst = ctx.enter_context(tc.tile_pool(name="pst", bufs=2, space="PSUM"))

            dma_a = nc.sync.dma_start      # queue A: PE-critical operands
            dma_b = nc.scalar.dma_start    # queue B: small tensors
            mm = nc.tensor.matmul

            # ---- persistent operands; DMAs chunked + ordered by first use ----
            xt = consts.tile([128, DPC, KT, L], fp8)     # d-major
            wrel = consts.tile([128, 2, KT, 512], fp8)   # h2-major
            wr1 = consts.tile([128, KT, H], fp8)
            at = consts.tile([128, DPC, R, 2, AW], fp8)
            bt = consts.tile([128, DPC, 2, L], fp8)
            wt = consts.tile([128, MT, MTP, 128], fp8)   # n2-major
            wlin = consts.tile([128, MTP, H], fp8)

            XTW = KT * L
            def dma_xt(d):
                dma_a(out=xt[:, d], in_=xt_d[:, d * XTW:(d + 1) * XTW])

            dma_a(out=wrel[:, 0, 0:4, :], in_=wrel_d[:, 0:2048])
            dma_xt(0)
            dma_a(out=wrel[:, 1, 0:4, :], in_=wrel_d[:, 4096:6144])
            dma_a(out=wrel[:, 0, 4:8, :], in_=wrel_d[:, 2048:4096])
            dma_a(out=wrel[:, 1, 4:8, :], in_=wrel_d[:, 6144:8192])
            dma_xt(1)
            dma_a(out=wr1, in_=wr1_d[:])
            dma_xt(2)
            dma_xt(3)
            ATW = R * 2 * AW
            for d in range(2):
                dma_a(out=at[:, d], in_=at_d[:, d * ATW:(d + 1) * ATW])
            dma_a(out=bt, in_=bt_d[:])
            for c0 in range(0, MT, 2):    # wt n2-pair chunks (4x2 + 1)
                c1 = min(c0 + 2, MT)
                dma_a(out=wt[:, c0:c1], in_=wt_d[:, c0 * MTP * 128:
                                              c1 * MTP * 128])
            for d in range(2, DPC):
                dma_a(out=at[:, d], in_=at_d[:, d * ATW:(d + 1) * ATW])
            dma_a(out=wlin, in_=wlin_d[:])
            w2 = consts.tile([128, 2, H], fp8)
            dma_b(out=w2, in_=w2_d[:])
            wfc = consts.tile([128, C], bf16)
            dma_b(out=wfc, in_=wfc_d[:])
            bias = consts.tile([128, 12], f32)
            dma_b(out=bias, in_=bias_d[:])
            bfc = consts.tile([1, C], bf16)
            dma_b(out=bfc, in_=bfc_d[:])
            if use_mask:
                import concourse.bass as bass
                um = consts.tile([128, DPC, 2, L], f32)
                src = um_d[:]
                bc = bass.AP(tensor=src.tensor, offset=src.offset,
                             ap=[[0, 128]] + list(src.ap))
                nc.gpsimd.dma_start(out=um, in_=bc)

            # small constants + memsets on Pool (ACT/DVE stay free)
            wz = consts.tile([128, 2, 512], fp8)
            nc.gpsimd.memset(wz, 0.0)
            ones_row = consts.tile([1, 128], bf16)
            nc.gpsimd.memset(ones_row, 1.0)
            neg1 = consts.tile([128, 1], f32)
            nc.gpsimd.memset(neg1, -1.0)
            ident = consts.tile([128, 128], bf16)
            make_identity(nc, ident)

            # persistent activation/state tiles (slot-major for d-contiguity)
            xr = consts.tile([128, NT, R, H], fp8)       # 16 * xr
            out1T = consts.tile([128, DPC, L], bf16)
            out18 = consts.tile([128, NT, H], fp8)
            nbout = consts.tile([128, 2, DPC, L], fp8)   # [nbT; out1T]
            out2x = consts.tile([128, 2, DPC, L], fp8)   # [out2T; ONES]
            hidT = consts.tile([128, DPC, L], bf16)
            G8 = consts.tile([128, DPC, 2, H], fp8)
            alphaT = consts.tile([128, DPC, 2, 2 * 128], fp8)  # 64*alpha^T
            s_all = consts.tile([128, 16], f32)
            o_all = consts.tile([128, DPC, 2, 8], f32)
            v_t = consts.tile([128, 16], f32)
            h_t = consts.tile([128, 16], f32)
            # ones row of M^T: pairs the 64*b_t row of w_t (bias via matmul)
            nc.gpsimd.memset(out2x[:, 1], 1.0)

            XcTs = []
            for d in range(DPC):
                XcT = big.tile([128, MTP, L], fp8, tag=f"XcT{d}")
                nc.gpsimd.memset(XcT[:, MT, :], 0.0)
                XcTs.append(XcT)

            # warm-up: dependency-free DR matmuls keep the PE p-state ramped
            # during the DMA lead-in; `warm` psum is never read.
            warm = ps.tile([128, 512], f32, tag="mm")
            for _ in range(N_WARM):
                mm(warm[:, :256], lhsT=wz[:, :, :128], rhs=wz[:, :, :256],
                   start=True, stop=True, perf_mode=DR, skip_group_check=True)

            # psum->SBUF copy on ACT (eng 0) or DVE (eng 1)
            def e_copy(eng, out, in_, scale=None):
                if eng == 0:
                    nc.scalar.activation(out, in_, AF.Identity,
                                         scale=(scale or 1.0))
                elif scale is not None:
                    nc.vector.tensor_scalar_mul(out, in_, scale)
                else:
                    nc.vector.tensor_copy(out, in_)

            # M^T subtile-pair accessor (m2 in 0..4): xt pairs, then
            # [out2T; ones]
            def rhs_pair(m2, d):
                if m2 < 4:
                    return xt[:, d, 2 * m2:2 * m2 + 2, :]
                return out2x[:, :, d, :]

            def lhs_pair(m2, d, st):
                if m2 < 4:
                    return xt[:, d, 2 * m2:2 * m2 + 2,
                              st * 128:(st + 1) * 128]
                return out2x[:, :, d, st * 128:st * 128 + 128]

            # ---- stage 1: xr = x @ w_rel; chain-major, per-dialogue waves --
            ci = 0
            for d in range(DPC):
                chains = []
                for h2 in range(2):
                    for st in range(2):
                        p_c = ps.tile([128, 512], f32, tag="mm")
                        chains.append((h2, st, p_c))
                for h2, st, p in chains:
                    for k2 in range(4):
                        mm(p, lhsT=xt[:, d, 2 * k2:2 * k2 + 2,
                                      st * 128:(st + 1) * 128],
                           rhs=wrel[:, h2, 2 * k2:2 * k2 + 2, :],
                           start=(k2 == 0), stop=(k2 == 3), perf_mode=DR,
                           skip_group_check=True)
                for h2, st, p in chains:
                    i = 2 * d + st
                    e_copy(ci % 2, xr[:, i, 4 * h2:4 * h2 + 4, :], p,
                           scale=1.0 / 16.0)
                    ci += 1

            def stage2_q(q):
                # root (DR, zeroes psum) + banded adjacency (relation-pair DR)
                pa = ps.tile([128, 512], f32, tag="mm")
                for j in range(2):
                    d = 2 * q + j
                    o = j * 256
                    for k2 in range(4):
                        mm(pa[:, o:o + L], lhsT=wr1[:, 2 * k2:2 * k2 + 2, :],
                           rhs=xt[:, d, 2 * k2:2 * k2 + 2, :],
                           start=(k2 == 0), stop=False, perf_mode=DR,
                           skip_group_check=True)
                    bi = 0
                    for ri in range(R // 2):
                        for st in range(2):
                            bi += 1
                            mm(pa[:, o + ALO[st]:o + ALO[st] + AW],
                               lhsT=xr[:, 2 * d + st, 2 * ri:2 * ri + 2, :],
                               rhs=at[:, d, 2 * ri:2 * ri + 2, st, :],
                               start=False, stop=(bi == R), perf_mode=DR,
                               skip_group_check=True)
                nc.scalar.activation(out1T[:, 2 * q:2 * q + 2, :], pa,
                                     AF.Identity, scale=1.0 / 64.0,
                                     bias=bias[:, 0:1])
                nc.gpsimd.tensor_copy(nbout[:, 1, 2 * q:2 * q + 2, :],
                                      out1T[:, 2 * q:2 * q + 2, :])
                for j in range(2):
                    d = 2 * q + j
                    tp = pst.tile([128, 256], bf16, tag="tr")
                    for st in range(2):
                        nc.tensor.transpose(
                            tp[:, st * 128:(st + 1) * 128],
                            out1T[:, d, st * 128:(st + 1) * 128], ident)
                    nc.vector.tensor_copy(out18[:, 2 * d:2 * d + 2, :], tp)

            def stage3_q(q):
                p2 = ps.tile([128, 512], f32, tag="mm")
                for j in range(2):
                    d = 2 * q + j
                    mm(p2[:, j * 256:j * 256 + L],
                       lhsT=out18[:, 2 * d:2 * d + 2, :],
                       rhs=bt[:, d, :, :], start=True, stop=True,
                       perf_mode=DR, skip_group_check=True)
                nc.vector.tensor_copy(nbout[:, 0, 2 * q:2 * q + 2, :], p2)
                p3 = ps.tile([128, 512], f32, tag="mm")
                for j in range(2):
                    d = 2 * q + j
                    mm(p3[:, j * 256:j * 256 + L], lhsT=w2,
                       rhs=nbout[:, :, d, :], start=True, stop=True,
                       perf_mode=DR, skip_group_check=True)
                nc.scalar.activation(out2x[:, 0, 2 * q:2 * q + 2, :], p3,
                                     AF.Identity, scale=1.0 / 64.0,
                                     bias=bias[:, 1:2])

            def stage5_d(d):
                # Xc^T = w_t^T M^T (+ b_t via ones row); paired n2 chains
                for n2p in range(4):
                    p4 = ps.tile([128, 512], f32, tag="mm")
                    for j in range(2):
                        n2 = 2 * n2p + j
                        for m2 in range(5):
                            mm(p4[:, j * 256:j * 256 + L],
                               lhsT=wt[:, n2, 2 * m2:2 * m2 + 2, :],
                               rhs=rhs_pair(m2, d), start=(m2 == 0),
                               stop=(m2 == 4), perf_mode=DR,
                               skip_group_check=True)
                    e_copy(0 if n2p == 0 else 1,
                           XcTs[d][:, 2 * n2p:2 * n2p + 2, :], p4)
                p4 = ps.tile([128, 512], f32, tag="mm")
                for m2 in range(5):
                    mm(p4[:, :L], lhsT=wt[:, MT - 1, 2 * m2:2 * m2 + 2, :],
                       rhs=rhs_pair(m2, d), start=(m2 == 0), stop=(m2 == 4),
                       perf_mode=DR)
                e_copy(0, XcTs[d][:, MT - 1, :], p4[:, :L])

            alfs = {}

            def scores_d(d):
                for tt in range(2):
                    p5 = ps.tile([128, 512], f32, tag="mm")
                    for n2 in range(5):
                        mm(p5[:, :L],
                           lhsT=XcTs[d][:, 2 * n2:2 * n2 + 2,
                                        tt * 128:(tt + 1) * 128],
                           rhs=rhs_pair(n2, d), start=(n2 == 0),
                           stop=(n2 == 4), perf_mode=DR)
                    z = big.tile([128, L], f32, tag=f"z{d}{tt}")
                    if use_mask:
                        # um slot0 = um^2/64 -> z_in = scores*um^2
                        nc.vector.tensor_mul(z, p5[:, :L], um[:, d, 0, :])
                        nc.scalar.activation(z, z, AF.Tanh)
                    else:
                        nc.scalar.activation(z, p5[:, :L], AF.Tanh,
                                             scale=1.0 / 64.0)
                    ssum = work.tile([128, 1], f32, tag=f"ssum{d}{tt}")
                    # tanh <= 1, so exp(z - 1) is safe without a max pass
                    nc.scalar.activation(z, z, AF.Exp, bias=neg1,
                                         accum_out=ssum)
                    if use_mask:
                        nc.vector.tensor_mul(z, z, um[:, d, 1, :])
                        nc.vector.reduce_sum(out=ssum, in_=z,
                                             axis=mybir.AxisListType.X)
                    rinv = work.tile([128, 1], f32, tag=f"rinv{d}{tt}")
                    nc.vector.reciprocal(rinv, ssum)
                    alf = big.tile([128, L], bf16, tag=f"alf{d}{tt}")
                    # 64*alpha on Pool (SBUF-only engine)
                    nc.gpsimd.tensor_scalar(out=alf, in0=z, scalar1=rinv,
                                            scalar2=64.0, op0=OP.mult,
                                            op1=OP.mult)
                    alfs[(d, tt)] = alf

            def g_d(d):
                pg = ps.tile([128, 512], f32, tag="mm")
                for st in range(2):
                    for m2 in range(5):
                        mm(pg[:, st * 128:(st + 1) * 128],
                           lhsT=lhs_pair(m2, d, st),
                           rhs=wlin[:, 2 * m2:2 * m2 + 2, :],
                           start=(m2 == 0), stop=(m2 == 4), perf_mode=DR,
                           skip_group_check=True)
                nc.vector.tensor_scalar_mul(G8[:, d, :, :], pg[:, :256],
                                            1.0 / 64.0)

            out_ap = out_d[:].rearrange("(d tt p) c -> p d tt c", d=DPC, tt=2)
            LN7 = float(np.log(7.0))

            def tail_d(d):
                # all four alpha transposes share one psum bank, one copy
                tp = pst.tile([128, 512], bf16, tag="tr")
                for st in range(2):
                    for tt in range(2):
                        nc.tensor.transpose(
                            tp[:, st * 256 + tt * 128:st * 256 + tt * 128 + 128],
                            alfs[(d, tt)][:, st * 128:(st + 1) * 128], ident)
                nc.vector.tensor_copy(alphaT[:, d, :, :], tp)
                p7 = ps.tile([128, 512], f32, tag="mm")
                mm(p7[:, :L], lhsT=G8[:, d, :, :], rhs=alphaT[:, d, :, :],
                   start=True, stop=True, perf_mode=DR)
                nc.scalar.activation(hidT[:, d, :], p7[:, :L], AF.Relu,
                                     scale=1.0 / 64.0, bias=bias[:, 11:12])
                p8 = ps.tile([128, 512], f32, tag="mm")
                for tt in range(2):
                    o = tt * 8
                    mm(p8[:, o:o + C],
                       lhsT=hidT[:, d, tt * 128:(tt + 1) * 128],
                       rhs=wfc, start=True, stop=False,
                       skip_group_check=True)
                    mm(p8[:, o:o + C], lhsT=ones_row, rhs=bfc, start=False,
                       stop=True, skip_group_check=True)
                for tt in range(2):
                    idx = d * 2 + tt
                    e8 = work.tile([128, 8], f32, tag="e8")
                    nc.scalar.activation(e8[:, :C], p8[:, tt * 8:tt * 8 + C],
                                         AF.Exp,
                                         accum_out=s_all[:, idx:idx + 1])
                nc.vector.tensor_copy(o_all[:, d, :, :], p8[:, :16])
                # per-dialogue log-softmax close-out + output DMA (Pool):
                # ln(s) = ln7 + v - v^2/2 + v^3/3, v = s/7 - 1 (|v| < 0.1)
                i0 = 2 * d
                sv = v_t[:, i0:i0 + 2]
                sh = h_t[:, i0:i0 + 2]
                nc.gpsimd.tensor_scalar(out=sv, in0=s_all[:, i0:i0 + 2],
                                        scalar1=1.0 / 7.0, scalar2=-1.0,
                                        op0=OP.mult, op1=OP.add)
                nc.gpsimd.tensor_scalar(out=sh, in0=sv, scalar1=1.0 / 3.0,
                                        scalar2=-0.5, op0=OP.mult, op1=OP.add)
                nc.gpsimd.tensor_mul(sh, sh, sv)
                nc.gpsimd.tensor_scalar_add(sh, sh, 1.0)
                nc.gpsimd.tensor_mul(sh, sh, sv)   # h = ln(s) - ln7
                for tt in range(2):
                    idx = d * 2 + tt
                    nc.gpsimd.tensor_scalar(
                        out=o_all[:, d, tt, :C], in0=o_all[:, d, tt, :C],
                        scalar1=h_t[:, idx:idx + 1], scalar2=-LN7,
                        op0=OP.subtract, op1=OP.add)
                dma_a(out=out_ap[:, d], in_=o_all[:, d, :, 0:C])

            # ---- pipelined back half over dialogue pairs; scores-d emitted
            # after G-d / tail-(d-1) so XcT copies hide under PE work ----
            for q in range(2):
                stage2_q(q)
                stage3_q(q)
                for d in (2 * q, 2 * q + 1):
                    stage5_d(d)
                    g_d(d)
                    scores_d(d)
                    if d > 0:
                        tail_d(d - 1)
            tail_d(DPC - 1)

    nc.compile()
    return nc


def prep_inputs(x, edge_src, edge_dst, edge_type, umask, basis, comp,
                w_root1, b1, w_rel2, b_rel2, w_root2, w_t, b_t,
                w_lin, b_lin, w_fc, b_fc):
    """Host-side sharding / packing into the device's [128, ...] layouts."""
    x = np.asarray(x, np.float32)
    src = np.asarray(edge_src, np.int64)
    dst = np.asarray(edge_dst, np.int64)
    ety = np.asarray(edge_type, np.int64)
    umask = np.asarray(umask, np.float32)
    basis = np.asarray(basis, np.float32)
    comp = np.asarray(comp, np.float32)

    g_s = src // L
    assert np.array_equal(g_s, dst // L), "edges must stay within a dialogue"

    # w_rel[r] = sum_b comp[r,b] basis[b]; packed h2-major:
    # wrel[p, h2, k, j] = 256 * w_rel[r, 128k+p, h], (r, h) = divmod(512*h2+j, H)
    w_rel = np.einsum('rb,bdh->rdh', comp, basis)          # (R, D, H)
    wr = (256.0 * w_rel).transpose(1, 0, 2).reshape(D, R * H)   # [d, rH]
    wr = wr.reshape(KT, 128, 2, 512).transpose(1, 2, 0, 3)      # [p, h2, k, 512]
    wrel_pack = np.ascontiguousarray(wr.reshape(128, 2 * KT * 512)).astype(FP8)

    deg = np.bincount(dst, minlength=N).astype(np.float64)
    inv_deg = np.where(deg > 0, 1.0 / np.maximum(deg, 1), 0.0).astype(np.float32)

    at_all = np.zeros((B, R, L, L), np.float32)   # [dlg, r, src, dst]
    ls, ld = src % L, dst % L
    np.add.at(at_all, (g_s, ety, ls, ld), 1.0)
    bt_all = np.zeros((B, L, L), np.float32)
    np.add.at(bt_all, (g_s, ls, ld), 1.0)
    # fold 4/deg into the relational masks (device pa = 64*(agg+root))
    at_all *= 4.0 * inv_deg.reshape(B, 1, 1, L)

    # band-crop: src tile st covers dst cols [ALO[st], ALO[st]+AW)
    at_band = np.zeros((B, R, 2, 128, AW), np.float32)
    for st in range(2):
        at_band[:, :, st] = at_all[:, :, st * 128:(st + 1) * 128,
                                   ALO[st]:ALO[st] + AW]
    assert np.isclose(at_band.sum(), at_all.sum()), \
        "edges outside the adjacency band"

    use_mask = not bool(np.all(umask == 1.0))

    bias_pack = np.zeros((128, 12), np.float32)
    bias_pack[:, 0] = np.asarray(b1, np.float32)
    bias_pack[:, 1] = np.asarray(b_rel2, np.float32)
    bias_pack[:, 11] = np.asarray(b_lin, np.float32)

    def pack_k(w, scale):   # [K*128, F] -> [128, KT', F]
        k = w.shape[0] // 128
        return np.ascontiguousarray(
            (scale * np.asarray(w, np.float32)).reshape(k, 128, -1)
            .transpose(1, 0, 2).reshape(128, -1)).astype(FP8)

    # wt n2-major: wt[p, n2, m, j] = 64*w_t[128m+p, 128n2+j];
    # row 1152 (m=9, p=0) = 64*b_t, pairing the ones row of M^T
    wt_pad = np.zeros((MTP * 128, MEM), np.float32)
    wt_pad[:MEM] = 64.0 * np.asarray(w_t, np.float32)
    wt_pad[MEM] = 64.0 * np.asarray(b_t, np.float32)
    wt9 = wt_pad.reshape(MTP, 128, MT, 128).transpose(1, 2, 0, 3)
    wt_pack = np.ascontiguousarray(wt9.reshape(128, -1)).astype(FP8)

    wlin_pad = np.zeros((MTP * 128, H), np.float32)
    wlin_pad[:MEM] = np.asarray(w_lin, np.float32)
    w2_stack = np.stack([np.asarray(w_rel2, np.float32),
                         np.asarray(w_root2, np.float32)], axis=1)  # [H,2,H]

    shared = {
        "wrel": wrel_pack,
        "wr1": pack_k(np.asarray(w_root1, np.float32), 64.0),
        "w2": np.ascontiguousarray(
            (64.0 * w2_stack).reshape(128, 2 * H)).astype(FP8),
        "wt": wt_pack,
        "wlin": pack_k(wlin_pad, 64.0),
        "wfc": np.asarray(w_fc, np.float32).astype(BF16),
        "bias": bias_pack,
        "bfc": np.asarray(b_fc, np.float32).reshape(1, C).astype(BF16),
    }

    in_maps = []
    for c in range(NCORES):
        m = dict(shared)
        xl = x[c * NLOC:(c + 1) * NLOC]           # (1024, 1024)
        # xt d-major: xt[p, d, k, j] = x[d*256+j, 128k+p]
        xtd = xl.T.reshape(KT, 128, DPC, L).transpose(1, 2, 0, 3)
        m["xt"] = np.ascontiguousarray(xtd.reshape(128, -1)).astype(FP8)
        atc = at_band[c * DPC:(c + 1) * DPC]      # (DPC, R, 2, 128, AW)
        m["at"] = np.ascontiguousarray(
            atc.transpose(3, 0, 1, 2, 4).reshape(128, -1)).astype(FP8)
        btc = bt_all[c * DPC:(c + 1) * DPC]
        m["bt"] = np.ascontiguousarray(
            btc.reshape(DPC, 2, 128, L).transpose(2, 0, 1, 3)
            .reshape(128, -1)).astype(FP8)
        if use_mask:
            uml = umask[c * DPC:(c + 1) * DPC]    # (DPC, L)
            m["um"] = np.stack([uml * uml / 64.0, uml], axis=1
                               ).astype(np.float32)
        in_maps.append(m)
    return in_maps, use_mask


_last_results = None


def kernel(**inputs):
    global _last_results
    from concourse.bass_utils import run_bass_kernel_spmd

    in_maps, use_mask = prep_inputs(**inputs)
    if use_mask not in _cache:
        _cache[use_mask] = _build_program(use_mask)
    nc = _cache[use_mask]
    res = run_bass_kernel_spmd(nc, in_maps, core_ids=list(range(NCORES)))
    _last_results = res
    return np.concatenate([res.results[c]["out"] for c in range(NCORES)],
                          axis=0)
